# revision 39
# baseline (speedup 1.0000x reference)
"""Sliding-window attention (RoPE + QKV proj + windowed softmax attention + o_proj)
for Trainium2, SPMD over 8 NeuronCores.

Sharding: batch (2) x head-groups (4 groups of 4 heads) -> 8 cores.
Each core computes qkv for its 4 heads, windowed attention, and a partial
o_proj (its heads' columns of w_o); host sums the 4 partials per batch.

v2 design (vs f32r baseline):
- fp16 everywhere on the PE (1 cyc/row at any free size; products exact in
  f32 PSUM). Host converts x/w to fp16; rel err ~5e-4 vs 2e-2 tolerance.
- No DRAM spill: rotated q/k ([dh, S] layout) and v ([S, dh]) live in SBUF
  between phases (6 MB in fp16).
- Attention computes S^T = K^T-slices x Q-slices directly per 128x128 cell:
  no PE transposes, no full-width mask preloads (only 2 boundary cells per
  q-block get a mask matmul), no PSUM zeroing. exp() -> fp16 probs; softmax
  denominators via an all-ones matmul riding the same prob tiles; normalize
  on DVE at PV-evac time.
"""
import sys

sys.path.insert(0, "/opt/trn_rl_repo")

import numpy as np

B = 2
S = 2048
HIDDEN = 2048
N_HEADS = 16
DH = 128
WINDOW = 512
HPC = 4  # heads per core
N_CORES = 8
QKV_O = 3 * HPC * DH  # 1536
SCALE = 1.0 / np.sqrt(DH)
NEG = -30000.0  # fp16-safe -inf; exp() underflows to exactly 0

_CACHE = {}

# tunable knobs (sim A/B)
CFG = {
    "psq_bufs": 6, "psv_bufs": 2,
    "xt_extra": 12, "rope_bufs": 3, "tbl_bufs": 2,
    "score_bufs": 3, "pvs_bufs": 2, "p_bufs": 3, "rc_bufs": 3,
    "pipe": 2,
    "psc_bufs": 4, "ob_bufs": 6,
    "phases": "ABC",
    "dump": False,
    "wt_pair": False,  # load wT as 8 two-chunk DMAs
    "wt0_split": False,
    "interleave_heads": False,
    "mask_eng": "pe",  # pe: -inf preload matmuls; gpsimd: 0/1 mul post-exp
    "xt_eng": "scalar",  # queue for xt/table loads (sync|scalar)
    "v_eng": "scalar",  # engine for v16 psum evac (vector|scalar)
}


def _build_module(repeat=1, cfg=None):
    cfg = {**CFG, **(cfg or {})}
    import concourse.tile as tile
    from concourse import bacc, mybir
    from contextlib import ExitStack

    f32 = mybir.dt.float32
    f16 = mybir.dt.float16
    AF = mybir.ActivationFunctionType

    nc = bacc.Bacc("TRN2", target_bir_lowering=False, debug=False)

    xT = nc.declare_dram_parameter("xT", [HIDDEN, S], f16, isOutput=False)
    wT = nc.declare_dram_parameter("wT", [HIDDEN, QKV_O], f16, isOutput=False)
    woT = nc.declare_dram_parameter("woT", [HPC * DH, HIDDEN], f16, isOutput=False)
    cosq = nc.declare_dram_parameter("cosq", [DH, S], f32, isOutput=False)
    sinq = nc.declare_dram_parameter("sinq", [DH, S], f32, isOutput=False)
    cosk = nc.declare_dram_parameter("cosk", [DH, S], f32, isOutput=False)
    sink = nc.declare_dram_parameter("sink", [DH, S], f32, isOutput=False)
    mdiag_d = nc.declare_dram_parameter("mdiag", [128, 128], f16, isOutput=False)
    mwend_d = nc.declare_dram_parameter("mwend", [128, 128], f16, isOutput=False)
    m01d_d = nc.declare_dram_parameter("m01d", [128, 128], f16, isOutput=False)
    m01w_d = nc.declare_dram_parameter("m01w", [128, 128], f16, isOutput=False)
    idn_d = nc.declare_dram_parameter("idn", [128, 128], f16, isOutput=False)
    ones_d = nc.declare_dram_parameter("ones", [128, 128], f16, isOutput=False)
    out_d = nc.declare_dram_parameter("out", [S, HIDDEN], f16, isOutput=True)
    if cfg["dump"]:
        dbg_qh = nc.declare_dram_parameter("dbg_qh", [HPC * 128, S], f16, isOutput=True)
        dbg_kh = nc.declare_dram_parameter("dbg_kh", [HPC * 128, S], f16, isOutput=True)
        dbg_v = nc.declare_dram_parameter("dbg_v", [S, HPC * DH], f16, isOutput=True)
        dbg_ah = nc.declare_dram_parameter("dbg_ah", [HPC * 128, S], f16, isOutput=True)

    NKT = HIDDEN // 128  # 16 contraction chunks
    NSC = S // 512  # 4 sequence chunks
    NST = S // 128  # 16 sequence tiles

    with tile.TileContext(nc) as tc, ExitStack() as top:
        cpool = top.enter_context(tc.tile_pool(name="consts", bufs=1))
        mdiag = cpool.tile([128, 128], f16, tag="mdiag")
        mwend = cpool.tile([128, 128], f16, tag="mwend")
        m01d = cpool.tile([128, 128], f16, tag="m01d")
        m01w = cpool.tile([128, 128], f16, tag="m01w")
        idn = cpool.tile([128, 128], f16, tag="idn")
        ones = cpool.tile([128, 128], f16, tag="ones")
        consts_loaded = False

        def load_consts():
            nc.sync.dma_start(mdiag[:], mdiag_d[:])
            nc.sync.dma_start(mwend[:], mwend_d[:])
            nc.sync.dma_start(m01d[:], m01d_d[:])
            nc.sync.dma_start(m01w[:], m01w_d[:])
            nc.sync.dma_start(idn[:], idn_d[:])
            nc.sync.dma_start(ones[:], ones_d[:])

        # persistent qkv/attn tiles (SBUF-resident between phases)
        perpool = top.enter_context(tc.tile_pool(name="qkv", bufs=1))

        # SBUF pools hoisted out of the rep loop (ring across reps) so the
        # next rep's loads can prefetch while this rep's tail drains; PSUM
        # pools stay phase-scoped (banks must be reused across phases)
        wt_pool = top.enter_context(tc.tile_pool(name="wt", bufs=NKT))
        xt_pool = top.enter_context(
            tc.tile_pool(name="xt", bufs=NKT + cfg["xt_extra"])
        )
        tbl_pool = top.enter_context(tc.tile_pool(name="tbl", bufs=cfg["tbl_bufs"]))
        rope_pool = top.enter_context(
            tc.tile_pool(name="rope", bufs=cfg["rope_bufs"])
        )
        wo_pool = top.enter_context(tc.tile_pool(name="wo", bufs=HPC))
        p_pool = top.enter_context(tc.tile_pool(name="pp", bufs=cfg["p_bufs"]))
        rc_pool = top.enter_context(tc.tile_pool(name="rc", bufs=cfg["rc_bufs"]))
        ob_pool = top.enter_context(tc.tile_pool(name="ob", bufs=cfg["ob_bufs"]))

        for rep in range(repeat):
            qh = [perpool.tile([128, S], f16, tag=f"qh{h}", name=f"qh{h}")
                  for h in range(HPC)]
            kh = [perpool.tile([128, S], f16, tag=f"kh{h}", name=f"kh{h}")
                  for h in range(HPC)]
            v16 = [perpool.tile([128, HPC * DH], f16, tag=f"v{j}", name=f"v{j}")
                   for j in range(NST)]
            ah = [perpool.tile([128, S], f16, tag=f"ah{h}", name=f"ah{h}")
                  for h in range(HPC)]

            # ------------- Phase A: QKV projection + RoPE -------------
            if "A" in cfg["phases"]:
              with ExitStack() as ph:
                # single psum ring shared by q/k chains and v chains
                ps_pool = ph.enter_context(
                    tc.tile_pool(name="psa", bufs=8, space="PSUM")
                )

                xt_dma = getattr(nc, cfg["xt_eng"]).dma_start
                # wt on the SP queue (paired 2-chunk DMAs), xt + tables on the
                # Act queue: parallel issue so sc=0's k-interleaved waves can
                # start as soon as chunk 0 lands
                wt_tiles = []
                xt0_tiles = []
                if cfg["wt_pair"]:
                    for kp in range(NKT // 2):
                        t = wt_pool.tile(
                            [128, 2 * QKV_O], f16, tag="wt2", bufs=NKT // 2
                        )
                        src = wT[kp * 256 : (kp + 1) * 256, :]
                        nc.sync.dma_start(
                            t[:].rearrange("p (a f) -> p a f", a=2),
                            src.rearrange("(a p) f -> p a f", p=128),
                        )
                        wt_tiles.append(t[:, 0:QKV_O])
                        wt_tiles.append(t[:, QKV_O : 2 * QKV_O])
                else:
                    for k in range(NKT):
                        t = wt_pool.tile([128, QKV_O], f16, tag="wt")
                        if k == 0 and cfg["wt0_split"]:
                            # first matmul only needs cols 0:128; split so it
                            # can start before the whole chunk lands
                            nc.sync.dma_start(
                                t[:, 0:128], wT[0:128, 0:128]
                            )
                            nc.sync.dma_start(
                                t[:, 128:QKV_O], wT[0:128, 128:QKV_O]
                            )
                        else:
                            nc.sync.dma_start(t[:], wT[k * 128 : (k + 1) * 128, :])
                        wt_tiles.append(t)
                for k in range(NKT):
                    x = xt_pool.tile([128, 512], f16, tag="xt")
                    xt_dma(x[:], xT[k * 128 : (k + 1) * 128, 0:512])
                    xt0_tiles.append(x)
                if rep == 0 and not consts_loaded:
                    load_consts()
                    consts_loaded = True

                for sc in range(NSC):
                    s0 = sc * 512
                    if sc == 0:
                        xt_tiles = xt0_tiles
                    else:
                        xt_tiles = []
                        for k in range(NKT):
                            t = xt_pool.tile([128, 512], f16, tag="xt")
                            xt_dma(
                                t[:], xT[k * 128 : (k + 1) * 128, s0 : s0 + 512]
                            )
                            xt_tiles.append(t)

                    tb = {}
                    for nm, src in (
                        ("cosq", cosq),
                        ("sinq", sinq),
                        ("cosk", cosk),
                        ("sink", sink),
                    ):
                        t = tbl_pool.tile([128, 512], f32, tag=nm)
                        xt_dma(t[:], src[:, s0 : s0 + 512])
                        tb[nm] = t

                    def rope_evac(t_o, ps):
                        ct = tb["cosq"] if t_o < HPC else tb["cosk"]
                        st = tb["sinq"] if t_o < HPC else tb["sink"]
                        tmp = rope_pool.tile([128, 512], f32, tag="tmp")
                        nc.vector.tensor_mul(tmp[0:64, :], ps[64:128, :], st[0:64, :])
                        nc.vector.tensor_mul(
                            tmp[64:128, :], ps[0:64, :], st[64:128, :]
                        )
                        qc = rope_pool.tile([128, 512], f32, tag="qc")
                        nc.vector.tensor_mul(qc[:], ps[:], ct[:])
                        dst = qh[t_o] if t_o < HPC else kh[t_o - HPC]
                        nc.vector.tensor_add(dst[:, s0 : s0 + 512], qc[:], tmp[:])

                    qk_ps = [
                        ps_pool.tile([128, 512], f32, tag="ps", name=f"psqk{sc}_{t}")
                        for t in range(2 * HPC)
                    ]
                    # k-interleaved waves only for the cold start (rep 0, sc 0);
                    # later reps have everything prefetched and chain-sequential
                    # emission staggers the rope evacs (no v-wave stall)
                    if sc == 0 and rep == 0:
                        # k-interleaved: consume each (wt, xt) chunk across all
                        # 8 chains as the DMA delivers it
                        for k in range(NKT):
                            last = k == NKT - 1
                            for t_o in range(2 * HPC):
                                nc.tensor.matmul(
                                    qk_ps[t_o][:],
                                    wt_tiles[k][:, t_o * 128 : (t_o + 1) * 128],
                                    xt_tiles[k][:],
                                    start=(k == 0),
                                    stop=last,
                                )
                                if last:
                                    rope_evac(t_o, qk_ps[t_o])
                    else:
                        # steady state: chain-sequential, evacs stagger
                        for t_o in range(2 * HPC):
                            for k in range(NKT):
                                nc.tensor.matmul(
                                    qk_ps[t_o][:],
                                    wt_tiles[k][:, t_o * 128 : (t_o + 1) * 128],
                                    xt_tiles[k][:],
                                    start=(k == 0),
                                    stop=(k == NKT - 1),
                                )
                            rope_evac(t_o, qk_ps[t_o])
                    for st_i in range(4):
                        psv = ps_pool.tile(
                            [128, 512], f32, tag="ps", name=f"psv{sc}_{st_i}"
                        )
                        for k in range(NKT):
                            nc.tensor.matmul(
                                psv[:],
                                xt_tiles[k][:, st_i * 128 : (st_i + 1) * 128],
                                wt_tiles[k][:, 2 * HPC * 128 : 3 * HPC * 128],
                                start=(k == 0),
                                stop=(k == NKT - 1),
                            )
                        if cfg["v_eng"] == "scalar":
                            nc.scalar.copy(v16[sc * 4 + st_i][:], psv[:])
                        else:
                            nc.vector.tensor_copy(v16[sc * 4 + st_i][:], psv[:])

            # ------------- Phase B: windowed attention (S^T cells) -------
            if "B" in cfg["phases"]:
              with ExitStack() as ph:
                wo_tiles = []
                for h in range(HPC):
                    t = wo_pool.tile([128, HIDDEN], f16, tag="wo")
                    nc.sync.dma_start(t[:], woT[h * 128 : (h + 1) * 128, :])
                    wo_tiles.append(t)

                phps = ExitStack()
                score_pool = phps.enter_context(
                    tc.tile_pool(name="score", bufs=cfg["score_bufs"], space="PSUM")
                )
                pvs_pool = phps.enter_context(
                    tc.tile_pool(name="pvs", bufs=cfg["pvs_bufs"], space="PSUM")
                )

                def emit_qk(h, i):
                    """Score cells S^T[j, q] for q-block i, all window j-blocks.

                    PSUM start=True pending-zeroes the whole 2KB bank, so each
                    bank gets exactly ONE start (its first matmul); later cells
                    in the bank are zeroed lazily on first touch.
                    """
                    j0 = max(0, i - 4)
                    nblk = i - j0 + 1
                    ps = score_pool.tile([128, 640], f32, tag="score")
                    qsl = qh[h][:, i * 128 : (i + 1) * 128]
                    use_pe_mask = cfg["mask_eng"] == "pe"
                    started = [False, False]  # bank 0: cells 0-3, bank 1: cell 4
                    for z in range(nblk):
                        jb = j0 + z
                        cell = ps[:, z * 128 : (z + 1) * 128]
                        bk = z // 4
                        last_in_bank = z == nblk - 1 or (z == 3 and nblk > 4)
                        msk = None
                        if jb == i:
                            msk = mdiag
                        elif jb == i - 4:
                            msk = mwend
                        if use_pe_mask and msk is not None:
                            nc.tensor.matmul(
                                cell, idn[:], msk[:],
                                start=not started[bk], stop=False,
                                skip_group_check=True,
                            )
                            started[bk] = True
                        nc.tensor.matmul(
                            cell,
                            kh[h][:, jb * 128 : (jb + 1) * 128],
                            qsl,
                            start=not started[bk],
                            stop=last_in_bank,
                            skip_group_check=True,
                        )
                        started[bk] = True
                    return ps, nblk, j0

                def emit_exp(h, i, ps, nblk, j0):
                    w = nblk * 128
                    pt = p_pool.tile([128, 640], f16, tag="p")
                    nc.scalar.activation(pt[:, :w], ps[:, :w], AF.Exp)
                    if cfg["mask_eng"] == "gpsimd":
                        # zero the forbidden triangles on the idle Pool engine
                        for z in range(nblk):
                            jb = j0 + z
                            m01 = None
                            if jb == i:
                                m01 = m01d
                            elif jb == i - 4:
                                m01 = m01w
                            if m01 is not None:
                                cell = pt[:, z * 128 : (z + 1) * 128]
                                nc.gpsimd.tensor_mul(cell, cell, m01[:])
                    return pt

                def emit_pv(h, i, pt, nblk, j0):
                    # PV accum (cols 0:128) and softmax sums (cols 128:256)
                    # share one bank: single start on the first matmul only.
                    pvs = pvs_pool.tile([128, 256], f32, tag="pvs")
                    for z in range(nblk):
                        jb = j0 + z
                        psl = pt[:, z * 128 : (z + 1) * 128]
                        nc.tensor.matmul(
                            pvs[:, 0:128],
                            v16[jb][:, h * 128 : (h + 1) * 128],
                            psl,
                            start=(z == 0), stop=False,
                            skip_group_check=True,
                        )
                        nc.tensor.matmul(
                            pvs[:, 128:256],
                            ones[:],
                            psl,
                            start=False, stop=(z == nblk - 1),
                            skip_group_check=True,
                        )
                    rc = rc_pool.tile([128, 128], f32, tag="rc")
                    nc.vector.reciprocal(rc[:], pvs[:, 128:256])
                    nc.vector.tensor_mul(
                        ah[h][:, i * 128 : (i + 1) * 128], pvs[:, 0:128], rc[:]
                    )

                if cfg["interleave_heads"]:
                    # tiny warm-up blocks (i<4) of head h+1 are slotted into
                    # the middle of head h's stream where the pipe is deep,
                    # avoiding a thin-pipe stall at every head boundary
                    blocks = [(0, i) for i in range(4)]
                    for h in range(HPC):
                        big = [(h, i) for i in range(4, NST)]
                        nxt = (
                            [(h + 1, i) for i in range(4)] if h + 1 < HPC else []
                        )
                        merged = []
                        for z, b in enumerate(big):
                            merged.append(b)
                            if z < len(nxt):
                                merged.append(nxt[z])
                        blocks += merged
                else:
                    blocks = [(h, i) for h in range(HPC) for i in range(NST)]
                pending = []
                for (h, i) in blocks:
                    ps, nblk, j0 = emit_qk(h, i)
                    pt = emit_exp(h, i, ps, nblk, j0)
                    pending.append((h, i, pt, nblk, j0))
                    if len(pending) > cfg["pipe"]:
                        emit_pv(*pending.pop(0))
                for it in pending:
                    emit_pv(*it)
                phps.close()

                if cfg["dump"]:
                    for h in range(HPC):
                        nc.sync.dma_start(dbg_qh[h * 128 : (h + 1) * 128, :], qh[h][:])
                        nc.sync.dma_start(dbg_kh[h * 128 : (h + 1) * 128, :], kh[h][:])
                        nc.sync.dma_start(dbg_ah[h * 128 : (h + 1) * 128, :], ah[h][:])
                    for j in range(NST):
                        nc.sync.dma_start(dbg_v[j * 128 : (j + 1) * 128, :], v16[j][:])

                # ------------- Phase C: output projection -------------
                if "C" in cfg["phases"]:
                    psc_pool = ph.enter_context(
                        tc.tile_pool(name="psc", bufs=cfg["psc_bufs"], space="PSUM")
                    )
                    for st_i in range(NST):
                        for mc in range(HIDDEN // 512):
                            ps = psc_pool.tile([128, 512], f32, tag="psc")
                            for h in range(HPC):
                                nc.tensor.matmul(
                                    ps[:],
                                    ah[h][:, st_i * 128 : (st_i + 1) * 128],
                                    wo_tiles[h][:, mc * 512 : (mc + 1) * 512],
                                    start=(h == 0),
                                    stop=(h == HPC - 1),
                                )
                            ob = ob_pool.tile([128, 512], f16, tag="ob")
                            nc.vector.tensor_copy(ob[:], ps[:])
                            nc.sync.dma_start(
                                out_d[
                                    st_i * 128 : (st_i + 1) * 128,
                                    mc * 512 : (mc + 1) * 512,
                                ],
                                ob[:],
                            )

    nc.compile()
    return nc


def _get_module(repeat=1, cfg=None):
    key = ("nc", repeat, tuple(sorted((cfg or {}).items())))
    if key not in _CACHE:
        _CACHE[key] = _build_module(repeat, cfg)
    return _CACHE[key]


def make_in_maps(hidden_states, cos, sin, w_qkv, w_o):
    hidden_states = np.asarray(hidden_states, dtype=np.float32)
    cos = np.asarray(cos, dtype=np.float32)
    sin = np.asarray(sin, dtype=np.float32)
    w_qkv = np.asarray(w_qkv, dtype=np.float32)
    w_o = np.asarray(w_o, dtype=np.float32)

    cosT = np.ascontiguousarray(cos.T)  # [DH, S]
    sinT = np.ascontiguousarray(sin.T)
    sinS = sinT.copy()
    sinS[: DH // 2] *= -1.0  # fold rotate_half sign
    cq = (cosT * SCALE).astype(np.float32)
    sq = (sinS * SCALE).astype(np.float32)
    ck = cosT.astype(np.float32)
    sk = sinS.astype(np.float32)

    # boundary-cell masks (in-cell coords: jj = key row, qq = query col)
    jj = np.arange(128)[:, None]
    qq = np.arange(128)[None, :]
    mdiag = np.where(qq >= jj, 0.0, NEG).astype(np.float16)
    mwend = np.where(qq < jj, 0.0, NEG).astype(np.float16)
    m01d = (qq >= jj).astype(np.float16)
    m01w = (qq < jj).astype(np.float16)
    idn = np.eye(128, dtype=np.float16)
    ones = np.ones((128, 128), dtype=np.float16)

    xTs = [np.ascontiguousarray(hidden_states[b].T).astype(np.float16)
           for b in range(B)]

    in_maps = []
    for c in range(N_CORES):
        b, hg = divmod(c, N_CORES // B)
        r0 = hg * HPC * DH
        wq = w_qkv[r0 : r0 + HPC * DH]
        wk = w_qkv[N_HEADS * DH + r0 : N_HEADS * DH + r0 + HPC * DH]
        wv = w_qkv[2 * N_HEADS * DH + r0 : 2 * N_HEADS * DH + r0 + HPC * DH]
        wTc = np.ascontiguousarray(
            np.concatenate([wq, wk, wv], axis=0).T
        ).astype(np.float16)
        woTc = np.ascontiguousarray(w_o[:, r0 : r0 + HPC * DH].T).astype(np.float16)
        in_maps.append(
            {
                "xT": xTs[b],
                "wT": wTc,
                "woT": woTc,
                "cosq": cq,
                "sinq": sq,
                "cosk": ck,
                "sink": sk,
                "mdiag": mdiag,
                "mwend": mwend,
                "m01d": m01d,
                "m01w": m01w,
                "idn": idn,
                "ones": ones,
            }
        )
    return in_maps


def gather(results):
    out = np.zeros((B, S, HIDDEN), dtype=np.float32)
    for c in range(N_CORES):
        b = c // (N_CORES // B)
        out[b] += results[c]["out"].astype(np.float32)
    return out


def kernel(hidden_states, cos, sin, w_qkv, w_o):
    from concourse.bass_utils import run_bass_kernel_spmd

    nc = _get_module()
    in_maps = make_in_maps(hidden_states, cos, sin, w_qkv, w_o)
    res = run_bass_kernel_spmd(nc, in_maps, list(range(N_CORES)))
    return gather(res.results)


# revision 41
# speedup vs baseline: 1.0088x; 1.0088x over previous
"""Sliding-window attention (RoPE + QKV proj + windowed softmax attention + o_proj)
for Trainium2, SPMD over 8 NeuronCores.

Sharding: batch (2) x head-groups (4 groups of 4 heads) -> 8 cores.
Each core computes qkv for its 4 heads, windowed attention, and a partial
o_proj (its heads' columns of w_o); host sums the 4 partials per batch.

v2 design (vs f32r baseline):
- fp16 everywhere on the PE (1 cyc/row at any free size; products exact in
  f32 PSUM). Host converts x/w to fp16; rel err ~5e-4 vs 2e-2 tolerance.
- No DRAM spill: rotated q/k ([dh, S] layout) and v ([S, dh]) live in SBUF
  between phases (6 MB in fp16).
- Attention computes S^T = K^T-slices x Q-slices directly per 128x128 cell:
  no PE transposes, no full-width mask preloads (only 2 boundary cells per
  q-block get a mask matmul), no PSUM zeroing. exp() -> fp16 probs; softmax
  denominators via an all-ones matmul riding the same prob tiles; normalize
  on DVE at PV-evac time.
"""
import sys

sys.path.insert(0, "/opt/trn_rl_repo")

import numpy as np

B = 2
S = 2048
HIDDEN = 2048
N_HEADS = 16
DH = 128
WINDOW = 512
HPC = 4  # heads per core
N_CORES = 8
QKV_O = 3 * HPC * DH  # 1536
SCALE = 1.0 / np.sqrt(DH)
NEG = -30000.0  # fp16-safe -inf; exp() underflows to exactly 0

_CACHE = {}

# tunable knobs (sim A/B)
CFG = {
    "psq_bufs": 6, "psv_bufs": 2,
    "xt_extra": 12, "rope_bufs": 3, "tbl_bufs": 2,
    "score_bufs": 3, "pvs_bufs": 2, "p_bufs": 3, "rc_bufs": 3,
    "pipe": 2,
    "psc_bufs": 4, "ob_bufs": 6,
    "phases": "ABC",
    "dump": False,
    "wt_pair": False,  # load wT as 8 two-chunk DMAs
    "wt0_split": False,
    "interleave_heads": False,
    "mask_eng": "pe",  # pe: -inf preload matmuls; gpsimd: 0/1 mul post-exp
    "xt_eng": "scalar",  # queue for xt/table loads (sync|scalar)
    "v_eng": "scalar",  # engine for v16 psum evac (vector|scalar)
    "ki_all_reps": False,  # k-interleave sc0 waves in every rep (not just rep 0)
}


def _build_module(repeat=1, cfg=None):
    cfg = {**CFG, **(cfg or {})}
    import concourse.tile as tile
    from concourse import bacc, mybir
    from contextlib import ExitStack

    f32 = mybir.dt.float32
    f16 = mybir.dt.float16
    AF = mybir.ActivationFunctionType

    nc = bacc.Bacc("TRN2", target_bir_lowering=False, debug=False)

    xT = nc.declare_dram_parameter("xT", [HIDDEN, S], f16, isOutput=False)
    wT = nc.declare_dram_parameter("wT", [HIDDEN, QKV_O], f16, isOutput=False)
    woT = nc.declare_dram_parameter("woT", [HPC * DH, HIDDEN], f16, isOutput=False)
    cosq = nc.declare_dram_parameter("cosq", [DH, S], f32, isOutput=False)
    sinq = nc.declare_dram_parameter("sinq", [DH, S], f32, isOutput=False)
    cosk = nc.declare_dram_parameter("cosk", [DH, S], f32, isOutput=False)
    sink = nc.declare_dram_parameter("sink", [DH, S], f32, isOutput=False)
    mdiag_d = nc.declare_dram_parameter("mdiag", [128, 128], f16, isOutput=False)
    mwend_d = nc.declare_dram_parameter("mwend", [128, 128], f16, isOutput=False)
    m01d_d = nc.declare_dram_parameter("m01d", [128, 128], f16, isOutput=False)
    m01w_d = nc.declare_dram_parameter("m01w", [128, 128], f16, isOutput=False)
    idn_d = nc.declare_dram_parameter("idn", [128, 128], f16, isOutput=False)
    ones_d = nc.declare_dram_parameter("ones", [128, 128], f16, isOutput=False)
    out_d = nc.declare_dram_parameter("out", [S, HIDDEN], f16, isOutput=True)
    if cfg["dump"]:
        dbg_qh = nc.declare_dram_parameter("dbg_qh", [HPC * 128, S], f16, isOutput=True)
        dbg_kh = nc.declare_dram_parameter("dbg_kh", [HPC * 128, S], f16, isOutput=True)
        dbg_v = nc.declare_dram_parameter("dbg_v", [S, HPC * DH], f16, isOutput=True)
        dbg_ah = nc.declare_dram_parameter("dbg_ah", [HPC * 128, S], f16, isOutput=True)

    NKT = HIDDEN // 128  # 16 contraction chunks
    NSC = S // 512  # 4 sequence chunks
    NST = S // 128  # 16 sequence tiles

    with tile.TileContext(nc) as tc, ExitStack() as top:
        cpool = top.enter_context(tc.tile_pool(name="consts", bufs=1))
        mdiag = cpool.tile([128, 128], f16, tag="mdiag")
        mwend = cpool.tile([128, 128], f16, tag="mwend")
        m01d = cpool.tile([128, 128], f16, tag="m01d")
        m01w = cpool.tile([128, 128], f16, tag="m01w")
        idn = cpool.tile([128, 128], f16, tag="idn")
        ones = cpool.tile([128, 128], f16, tag="ones")
        consts_loaded = False

        def load_consts():
            nc.sync.dma_start(mdiag[:], mdiag_d[:])
            nc.sync.dma_start(mwend[:], mwend_d[:])
            nc.sync.dma_start(m01d[:], m01d_d[:])
            nc.sync.dma_start(m01w[:], m01w_d[:])
            nc.sync.dma_start(idn[:], idn_d[:])
            nc.sync.dma_start(ones[:], ones_d[:])

        # persistent qkv/attn tiles (SBUF-resident between phases)
        perpool = top.enter_context(tc.tile_pool(name="qkv", bufs=1))

        # SBUF pools hoisted out of the rep loop (ring across reps) so the
        # next rep's loads can prefetch while this rep's tail drains; PSUM
        # pools stay phase-scoped (banks must be reused across phases)
        wt_pool = top.enter_context(tc.tile_pool(name="wt", bufs=NKT))
        xt_pool = top.enter_context(
            tc.tile_pool(name="xt", bufs=NKT + cfg["xt_extra"])
        )
        tbl_pool = top.enter_context(tc.tile_pool(name="tbl", bufs=cfg["tbl_bufs"]))
        rope_pool = top.enter_context(
            tc.tile_pool(name="rope", bufs=cfg["rope_bufs"])
        )
        wo_pool = top.enter_context(tc.tile_pool(name="wo", bufs=HPC))
        p_pool = top.enter_context(tc.tile_pool(name="pp", bufs=cfg["p_bufs"]))
        rc_pool = top.enter_context(tc.tile_pool(name="rc", bufs=cfg["rc_bufs"]))
        ob_pool = top.enter_context(tc.tile_pool(name="ob", bufs=cfg["ob_bufs"]))

        for rep in range(repeat):
            qh = [perpool.tile([128, S], f16, tag=f"qh{h}", name=f"qh{h}")
                  for h in range(HPC)]
            kh = [perpool.tile([128, S], f16, tag=f"kh{h}", name=f"kh{h}")
                  for h in range(HPC)]
            v16 = [perpool.tile([128, HPC * DH], f16, tag=f"v{j}", name=f"v{j}")
                   for j in range(NST)]
            ah = [perpool.tile([128, S], f16, tag=f"ah{h}", name=f"ah{h}")
                  for h in range(HPC)]

            # ------------- Phase A: QKV projection + RoPE -------------
            if "A" in cfg["phases"]:
              with ExitStack() as ph:
                # single psum ring shared by q/k chains and v chains
                ps_pool = ph.enter_context(
                    tc.tile_pool(name="psa", bufs=8, space="PSUM")
                )

                xt_dma = getattr(nc, cfg["xt_eng"]).dma_start
                # wt on the SP queue (paired 2-chunk DMAs), xt + tables on the
                # Act queue: parallel issue so sc=0's k-interleaved waves can
                # start as soon as chunk 0 lands
                wt_tiles = []
                xt0_tiles = []
                if cfg["wt_pair"]:
                    for kp in range(NKT // 2):
                        t = wt_pool.tile(
                            [128, 2 * QKV_O], f16, tag="wt2", bufs=NKT // 2
                        )
                        src = wT[kp * 256 : (kp + 1) * 256, :]
                        nc.sync.dma_start(
                            t[:].rearrange("p (a f) -> p a f", a=2),
                            src.rearrange("(a p) f -> p a f", p=128),
                        )
                        wt_tiles.append(t[:, 0:QKV_O])
                        wt_tiles.append(t[:, QKV_O : 2 * QKV_O])
                else:
                    for k in range(NKT):
                        t = wt_pool.tile([128, QKV_O], f16, tag="wt")
                        if k == 0 and cfg["wt0_split"]:
                            # first matmul only needs cols 0:128; split so it
                            # can start before the whole chunk lands
                            nc.sync.dma_start(
                                t[:, 0:128], wT[0:128, 0:128]
                            )
                            nc.sync.dma_start(
                                t[:, 128:QKV_O], wT[0:128, 128:QKV_O]
                            )
                        else:
                            nc.sync.dma_start(t[:], wT[k * 128 : (k + 1) * 128, :])
                        wt_tiles.append(t)
                for k in range(NKT):
                    x = xt_pool.tile([128, 512], f16, tag="xt")
                    xt_dma(x[:], xT[k * 128 : (k + 1) * 128, 0:512])
                    xt0_tiles.append(x)
                if rep == 0 and not consts_loaded:
                    load_consts()
                    consts_loaded = True

                for sc in range(NSC):
                    s0 = sc * 512
                    if sc == 0:
                        xt_tiles = xt0_tiles
                    else:
                        xt_tiles = []
                        for k in range(NKT):
                            t = xt_pool.tile([128, 512], f16, tag="xt")
                            xt_dma(
                                t[:], xT[k * 128 : (k + 1) * 128, s0 : s0 + 512]
                            )
                            xt_tiles.append(t)

                    tb = {}
                    for nm, src in (
                        ("cosq", cosq),
                        ("sinq", sinq),
                        ("cosk", cosk),
                        ("sink", sink),
                    ):
                        t = tbl_pool.tile([128, 512], f32, tag=nm)
                        xt_dma(t[:], src[:, s0 : s0 + 512])
                        tb[nm] = t

                    def rope_evac(t_o, ps):
                        ct = tb["cosq"] if t_o < HPC else tb["cosk"]
                        st = tb["sinq"] if t_o < HPC else tb["sink"]
                        tmp = rope_pool.tile([128, 512], f32, tag="tmp")
                        nc.vector.tensor_mul(tmp[0:64, :], ps[64:128, :], st[0:64, :])
                        nc.vector.tensor_mul(
                            tmp[64:128, :], ps[0:64, :], st[64:128, :]
                        )
                        qc = rope_pool.tile([128, 512], f32, tag="qc")
                        nc.vector.tensor_mul(qc[:], ps[:], ct[:])
                        dst = qh[t_o] if t_o < HPC else kh[t_o - HPC]
                        nc.vector.tensor_add(dst[:, s0 : s0 + 512], qc[:], tmp[:])

                    qk_ps = [
                        ps_pool.tile([128, 512], f32, tag="ps", name=f"psqk{sc}_{t}")
                        for t in range(2 * HPC)
                    ]
                    # k-interleaved waves only for the cold start (rep 0, sc 0);
                    # later reps have everything prefetched and chain-sequential
                    # emission staggers the rope evacs (no v-wave stall)
                    if sc == 0 and (rep == 0 or cfg["ki_all_reps"]):
                        # k-interleaved: consume each (wt, xt) chunk across all
                        # 8 chains as the DMA delivers it
                        for k in range(NKT):
                            last = k == NKT - 1
                            for t_o in range(2 * HPC):
                                nc.tensor.matmul(
                                    qk_ps[t_o][:],
                                    wt_tiles[k][:, t_o * 128 : (t_o + 1) * 128],
                                    xt_tiles[k][:],
                                    start=(k == 0),
                                    stop=last,
                                )
                                if last:
                                    rope_evac(t_o, qk_ps[t_o])
                    else:
                        # steady state: chain-sequential, evacs stagger
                        for t_o in range(2 * HPC):
                            for k in range(NKT):
                                nc.tensor.matmul(
                                    qk_ps[t_o][:],
                                    wt_tiles[k][:, t_o * 128 : (t_o + 1) * 128],
                                    xt_tiles[k][:],
                                    start=(k == 0),
                                    stop=(k == NKT - 1),
                                )
                            rope_evac(t_o, qk_ps[t_o])
                    for st_i in range(4):
                        psv = ps_pool.tile(
                            [128, 512], f32, tag="ps", name=f"psv{sc}_{st_i}"
                        )
                        for k in range(NKT):
                            nc.tensor.matmul(
                                psv[:],
                                xt_tiles[k][:, st_i * 128 : (st_i + 1) * 128],
                                wt_tiles[k][:, 2 * HPC * 128 : 3 * HPC * 128],
                                start=(k == 0),
                                stop=(k == NKT - 1),
                            )
                        if cfg["v_eng"] == "scalar":
                            nc.scalar.copy(v16[sc * 4 + st_i][:], psv[:])
                        else:
                            nc.vector.tensor_copy(v16[sc * 4 + st_i][:], psv[:])

            # ------------- Phase B: windowed attention (S^T cells) -------
            if "B" in cfg["phases"]:
              with ExitStack() as ph:
                wo_tiles = []
                for h in range(HPC):
                    t = wo_pool.tile([128, HIDDEN], f16, tag="wo")
                    nc.sync.dma_start(t[:], woT[h * 128 : (h + 1) * 128, :])
                    wo_tiles.append(t)

                phps = ExitStack()
                score_pool = phps.enter_context(
                    tc.tile_pool(name="score", bufs=cfg["score_bufs"], space="PSUM")
                )
                pvs_pool = phps.enter_context(
                    tc.tile_pool(name="pvs", bufs=cfg["pvs_bufs"], space="PSUM")
                )

                def emit_qk(h, i):
                    """Score cells S^T[j, q] for q-block i, all window j-blocks.

                    PSUM start=True pending-zeroes the whole 2KB bank, so each
                    bank gets exactly ONE start (its first matmul); later cells
                    in the bank are zeroed lazily on first touch.
                    """
                    j0 = max(0, i - 4)
                    nblk = i - j0 + 1
                    ps = score_pool.tile([128, 640], f32, tag="score")
                    qsl = qh[h][:, i * 128 : (i + 1) * 128]
                    use_pe_mask = cfg["mask_eng"] == "pe"
                    started = [False, False]  # bank 0: cells 0-3, bank 1: cell 4
                    for z in range(nblk):
                        jb = j0 + z
                        cell = ps[:, z * 128 : (z + 1) * 128]
                        bk = z // 4
                        last_in_bank = z == nblk - 1 or (z == 3 and nblk > 4)
                        msk = None
                        if jb == i:
                            msk = mdiag
                        elif jb == i - 4:
                            msk = mwend
                        if use_pe_mask and msk is not None:
                            nc.tensor.matmul(
                                cell, idn[:], msk[:],
                                start=not started[bk], stop=False,
                                skip_group_check=True,
                            )
                            started[bk] = True
                        nc.tensor.matmul(
                            cell,
                            kh[h][:, jb * 128 : (jb + 1) * 128],
                            qsl,
                            start=not started[bk],
                            stop=last_in_bank,
                            skip_group_check=True,
                        )
                        started[bk] = True
                    return ps, nblk, j0

                def emit_exp(h, i, ps, nblk, j0):
                    w = nblk * 128
                    pt = p_pool.tile([128, 640], f16, tag="p")
                    nc.scalar.activation(pt[:, :w], ps[:, :w], AF.Exp)
                    if cfg["mask_eng"] == "gpsimd":
                        # zero the forbidden triangles on the idle Pool engine
                        for z in range(nblk):
                            jb = j0 + z
                            m01 = None
                            if jb == i:
                                m01 = m01d
                            elif jb == i - 4:
                                m01 = m01w
                            if m01 is not None:
                                cell = pt[:, z * 128 : (z + 1) * 128]
                                nc.gpsimd.tensor_mul(cell, cell, m01[:])
                    return pt

                def emit_pv(h, i, pt, nblk, j0):
                    # PV accum (cols 0:128) and softmax sums (cols 128:256)
                    # share one bank: single start on the first matmul only.
                    pvs = pvs_pool.tile([128, 256], f32, tag="pvs")
                    for z in range(nblk):
                        jb = j0 + z
                        psl = pt[:, z * 128 : (z + 1) * 128]
                        nc.tensor.matmul(
                            pvs[:, 0:128],
                            v16[jb][:, h * 128 : (h + 1) * 128],
                            psl,
                            start=(z == 0), stop=False,
                            skip_group_check=True,
                        )
                        nc.tensor.matmul(
                            pvs[:, 128:256],
                            ones[:],
                            psl,
                            start=False, stop=(z == nblk - 1),
                            skip_group_check=True,
                        )
                    rc = rc_pool.tile([128, 128], f32, tag="rc")
                    nc.vector.reciprocal(rc[:], pvs[:, 128:256])
                    nc.vector.tensor_mul(
                        ah[h][:, i * 128 : (i + 1) * 128], pvs[:, 0:128], rc[:]
                    )

                if cfg["interleave_heads"]:
                    # tiny warm-up blocks (i<4) of head h+1 are slotted into
                    # the middle of head h's stream where the pipe is deep,
                    # avoiding a thin-pipe stall at every head boundary
                    blocks = [(0, i) for i in range(4)]
                    for h in range(HPC):
                        big = [(h, i) for i in range(4, NST)]
                        nxt = (
                            [(h + 1, i) for i in range(4)] if h + 1 < HPC else []
                        )
                        merged = []
                        for z, b in enumerate(big):
                            merged.append(b)
                            if z < len(nxt):
                                merged.append(nxt[z])
                        blocks += merged
                else:
                    blocks = [(h, i) for h in range(HPC) for i in range(NST)]
                pending = []
                for (h, i) in blocks:
                    ps, nblk, j0 = emit_qk(h, i)
                    pt = emit_exp(h, i, ps, nblk, j0)
                    pending.append((h, i, pt, nblk, j0))
                    if len(pending) > cfg["pipe"]:
                        emit_pv(*pending.pop(0))
                for it in pending:
                    emit_pv(*it)
                phps.close()

                if cfg["dump"]:
                    for h in range(HPC):
                        nc.sync.dma_start(dbg_qh[h * 128 : (h + 1) * 128, :], qh[h][:])
                        nc.sync.dma_start(dbg_kh[h * 128 : (h + 1) * 128, :], kh[h][:])
                        nc.sync.dma_start(dbg_ah[h * 128 : (h + 1) * 128, :], ah[h][:])
                    for j in range(NST):
                        nc.sync.dma_start(dbg_v[j * 128 : (j + 1) * 128, :], v16[j][:])

                # ------------- Phase C: output projection -------------
                if "C" in cfg["phases"]:
                    psc_pool = ph.enter_context(
                        tc.tile_pool(name="psc", bufs=cfg["psc_bufs"], space="PSUM")
                    )
                    for st_i in range(NST):
                        for mc in range(HIDDEN // 512):
                            ps = psc_pool.tile([128, 512], f32, tag="psc")
                            for h in range(HPC):
                                nc.tensor.matmul(
                                    ps[:],
                                    ah[h][:, st_i * 128 : (st_i + 1) * 128],
                                    wo_tiles[h][:, mc * 512 : (mc + 1) * 512],
                                    start=(h == 0),
                                    stop=(h == HPC - 1),
                                )
                            ob = ob_pool.tile([128, 512], f16, tag="ob")
                            nc.vector.tensor_copy(ob[:], ps[:])
                            nc.sync.dma_start(
                                out_d[
                                    st_i * 128 : (st_i + 1) * 128,
                                    mc * 512 : (mc + 1) * 512,
                                ],
                                ob[:],
                            )

    nc.compile()
    return nc


def _get_module(repeat=1, cfg=None):
    key = ("nc", repeat, tuple(sorted((cfg or {}).items())))
    if key not in _CACHE:
        _CACHE[key] = _build_module(repeat, cfg)
    return _CACHE[key]


def make_in_maps(hidden_states, cos, sin, w_qkv, w_o):
    hidden_states = np.asarray(hidden_states, dtype=np.float32)
    cos = np.asarray(cos, dtype=np.float32)
    sin = np.asarray(sin, dtype=np.float32)
    w_qkv = np.asarray(w_qkv, dtype=np.float32)
    w_o = np.asarray(w_o, dtype=np.float32)

    cosT = np.ascontiguousarray(cos.T)  # [DH, S]
    sinT = np.ascontiguousarray(sin.T)
    sinS = sinT.copy()
    sinS[: DH // 2] *= -1.0  # fold rotate_half sign
    cq = (cosT * SCALE).astype(np.float32)
    sq = (sinS * SCALE).astype(np.float32)
    ck = cosT.astype(np.float32)
    sk = sinS.astype(np.float32)

    # boundary-cell masks (in-cell coords: jj = key row, qq = query col)
    jj = np.arange(128)[:, None]
    qq = np.arange(128)[None, :]
    mdiag = np.where(qq >= jj, 0.0, NEG).astype(np.float16)
    mwend = np.where(qq < jj, 0.0, NEG).astype(np.float16)
    m01d = (qq >= jj).astype(np.float16)
    m01w = (qq < jj).astype(np.float16)
    idn = np.eye(128, dtype=np.float16)
    ones = np.ones((128, 128), dtype=np.float16)

    xTs = [np.ascontiguousarray(hidden_states[b].T).astype(np.float16)
           for b in range(B)]

    in_maps = []
    for c in range(N_CORES):
        b, hg = divmod(c, N_CORES // B)
        r0 = hg * HPC * DH
        wq = w_qkv[r0 : r0 + HPC * DH]
        wk = w_qkv[N_HEADS * DH + r0 : N_HEADS * DH + r0 + HPC * DH]
        wv = w_qkv[2 * N_HEADS * DH + r0 : 2 * N_HEADS * DH + r0 + HPC * DH]
        wTc = np.ascontiguousarray(
            np.concatenate([wq, wk, wv], axis=0).T
        ).astype(np.float16)
        woTc = np.ascontiguousarray(w_o[:, r0 : r0 + HPC * DH].T).astype(np.float16)
        in_maps.append(
            {
                "xT": xTs[b],
                "wT": wTc,
                "woT": woTc,
                "cosq": cq,
                "sinq": sq,
                "cosk": ck,
                "sink": sk,
                "mdiag": mdiag,
                "mwend": mwend,
                "m01d": m01d,
                "m01w": m01w,
                "idn": idn,
                "ones": ones,
            }
        )
    return in_maps


def gather(results):
    out = np.zeros((B, S, HIDDEN), dtype=np.float32)
    for c in range(N_CORES):
        b = c // (N_CORES // B)
        out[b] += results[c]["out"].astype(np.float32)
    return out


def kernel(hidden_states, cos, sin, w_qkv, w_o):
    from concourse.bass_utils import run_bass_kernel_spmd

    nc = _get_module()
    in_maps = make_in_maps(hidden_states, cos, sin, w_qkv, w_o)
    res = run_bass_kernel_spmd(nc, in_maps, list(range(N_CORES)))
    return gather(res.results)
